# revision 9
# baseline (speedup 1.0000x reference)
"""Trainium2 Bass kernel for a 12-head attention layer with RoPE + causal SDPA.

Problem shapes (hardcoded): B=4, S=2048, E=1152, H=12, D=96.

Sharding: 8 cores = 4 batches x 2 head-groups (6 heads each). Each core:
  - transposes its batch's logits to X^T on-chip (PE transpose)
  - computes Q^T, K^T (per-head, transposed layout) and V (natural layout,
    with a fused all-ones column per head for softmax denominators)
  - applies RoPE (head-dim permutation folded into Wq/Wk host-side so the
    interleaved-pair rotation becomes rotate-half form)
  - causal SDPA with scores in S^T [k, q] layout (softmax normalization via
    denominator row + gpsimd partition-broadcast at the ctx stage)
  - partial output projection over its 6 heads, interleaved per q-chunk
Host sums the two partials per batch and adds bo.

Dtypes: float32r (TF32-like) for projections & output proj, bf16 for
scores/probs/attn-V, fp32 accumulation in PSUM.
"""
import sys

sys.path.insert(0, "/opt/trn_rl_repo")

import numpy as np
import ml_dtypes
from contextlib import ExitStack

import concourse.bass as bass
import concourse.tile as tile
from concourse import bacc, mybir
from concourse.bass_utils import run_bass_kernel_spmd
from concourse.masks import make_identity

F32 = mybir.dt.float32
F32R = mybir.dt.float32r
BF16 = mybir.dt.bfloat16

B, S, E, H, D = 4, 2048, 1152, 12, 96
H6 = 6                    # heads per core
KC = E // 128             # 9 contraction chunks
TT = S // 128             # 16 token tiles
QCW = 512                 # query-chunk width
NQC = S // QCW            # 4 query chunks
SCALE = 1.0 / float(np.sqrt(D))
ROPE_BASE = 10000.0


def _emit(nc, tc, t, rep):
    """Emit one full kernel body. t = dict of DRAM APs."""
    with ExitStack() as top:
        persist = top.enter_context(tc.tile_pool(name=f"persist{rep}", bufs=1))
        qk_bf = {}
        for h in range(H6):
            qk_bf[("q", h)] = persist.tile([D, S], BF16, tag=f"qbf{h}", name=f"qbf{h}")
            qk_bf[("k", h)] = persist.tile([D, S], BF16, tag=f"kbf{h}", name=f"kbf{h}")
        v_sb = [persist.tile([128, H6 * 97], BF16, tag=f"v{i}", name=f"v{i}")
                for i in range(TT)]

        ident = persist.tile([128, 128], F32, tag="ident")
        make_identity(nc, ident[:])
        ones_f = persist.tile([1, 128], F32, tag="ones_f")
        nc.vector.memset(ones_f[:], 1.0)
        ones_r = persist.tile([1, 128], F32R, tag="ones_r")
        nc.vector.tensor_copy(ones_r[:], ones_f[:])

        bq_sb = persist.tile([D, H6], F32, tag="bq")
        nc.sync.dma_start(bq_sb[:], t["bq"])
        bk_sb = persist.tile([D, H6], F32, tag="bk")
        nc.sync.dma_start(bk_sb[:], t["bk"])

        # ================= Phase A: X^T, projections, RoPE =================
        with tc.tile_pool(name=f"pa{rep}", bufs=1) as pa, \
             tc.tile_pool(name=f"pa_ps{rep}", bufs=1, space="PSUM") as pa_ps:
            cos_sb = pa.tile([D, S], F32, tag="cos")
            nc.sync.dma_start(cos_sb[:], t["cosext"])
            sin_sb = pa.tile([D, S], F32, tag="sin")
            nc.sync.dma_start(sin_sb[:], t["sinext"])

            for hf in range(2):
                hw = S // 2
                t0 = hf * (TT // 2)
                # -- X^T for this half --
                xT = [pa.tile([128, hw], F32R, tag=f"xT{k}", name=f"xT{k}")
                      for k in range(KC)]
                for tl in range(TT // 2):
                    tt = t0 + tl
                    xn = pa.tile([128, E], F32, tag="xnat", bufs=2)
                    nc.sync.dma_start(xn[:], t["x"][tt * 128:(tt + 1) * 128, :])
                    for kg in range(3):
                        kcs = range(kg * 4, min(KC, kg * 4 + 4))
                        tp = pa_ps.tile([128, 128 * len(kcs)], F32, tag="trans", bufs=2)
                        for i, k in enumerate(kcs):
                            nc.tensor.transpose(
                                tp[:, i * 128:(i + 1) * 128],
                                xn[:, k * 128:(k + 1) * 128], ident[:])
                        for i, k in enumerate(kcs):
                            nc.scalar.copy(
                                xT[k][:, tl * 128:(tl + 1) * 128],
                                tp[:, i * 128:(i + 1) * 128])

                # -- V projection (natural layout, fused bias+ones row) --
                wv_sb = [pa.tile([128, H6 * 97], F32R, tag=f"wv{k}", name=f"wv{k}")
                         for k in range(KC)]
                for k in range(KC):
                    nc.sync.dma_start(wv_sb[k][:], t["wv"][k * 128:(k + 1) * 128, :])
                wv_b = pa.tile([1, H6 * 97], F32R, tag="wvb")
                nc.sync.dma_start(wv_b[:], t["wv"][E:E + 1, :])
                NB = (288, 294)  # psum bank split of 582 (f32r needs even N)
                for tl in range(TT // 2):
                    tt = t0 + tl
                    vps = [pa_ps.tile([128, n], F32, tag=f"vps{i}", bufs=2,
                                      name=f"vps{i}") for i, n in enumerate(NB)]
                    for k in range(KC + 1):
                        lhsT = (ones_r[:] if k == KC
                                else xT[k][:, tl * 128:(tl + 1) * 128])
                        c0 = 0
                        for i, n in enumerate(NB):
                            rhs = (wv_b[:, c0:c0 + n] if k == KC
                                   else wv_sb[k][:, c0:c0 + n])
                            nc.tensor.matmul(vps[i][:], lhsT, rhs,
                                             start=(k == 0), stop=(k == KC))
                            c0 += n
                    c0 = 0
                    for i, n in enumerate(NB):
                        nc.scalar.copy(v_sb[tt][:, c0:c0 + n], vps[i][:])
                        c0 += n

                # -- Q^T / K^T projections + RoPE (head pairs share W tiles) --
                for hp in range(H6 // 2):
                    for which, wname, b_sb in (("q", "wq", bq_sb), ("k", "wk", bk_sb)):
                        wp = [pa.tile([128, 2 * D], F32R, tag=f"wp{k}", bufs=2,
                                      name=f"wp{k}") for k in range(KC)]
                        for k in range(KC):
                            nc.sync.dma_start(
                                wp[k][:],
                                t[wname][k * 128:(k + 1) * 128,
                                         hp * 2 * D:(hp + 1) * 2 * D])
                        for hl in range(2):
                            h = 2 * hp + hl
                            raw = pa.tile([D, hw], F32, tag="qkraw", bufs=2)
                            for qc in range(2):
                                ps = pa_ps.tile([D, QCW], F32, tag="qkps", bufs=2)
                                for k in range(KC):
                                    nc.tensor.matmul(
                                        ps[:], wp[k][:, hl * D:(hl + 1) * D],
                                        xT[k][:, qc * QCW:(qc + 1) * QCW],
                                        start=(k == 0), stop=(k == KC - 1))
                                nc.scalar.add(
                                    raw[:, qc * QCW:(qc + 1) * QCW], ps[:],
                                    b_sb[:, h:h + 1])
                            # RoPE (rotate-half form after host-side permutation)
                            oc = qk_bf[(which, h)][:, hf * hw:(hf + 1) * hw]
                            swp = pa.tile([D, hw], F32, tag="swp", bufs=2)
                            nc.sync.dma_start(swp[0:48, :], raw[48:96, :])
                            nc.sync.dma_start(swp[48:96, :], raw[0:48, :])
                            nc.vector.tensor_mul(
                                oc, raw[:], cos_sb[:, hf * hw:(hf + 1) * hw])
                            tmp = pa.tile([D, hw], BF16, tag="ropetmp", bufs=2)
                            nc.vector.tensor_mul(
                                tmp[:], swp[:], sin_sb[:, hf * hw:(hf + 1) * hw])
                            nc.vector.tensor_add(oc, oc, tmp[:])

        # ============ Phase B+C: causal SDPA + output projection ============
        with tc.tile_pool(name=f"pb{rep}", bufs=1) as pb, \
             tc.tile_pool(name=f"pb_ps{rep}", bufs=1, space="PSUM") as pb_ps:
            ctx_sb = [pb.tile([D, S], F32R, tag=f"ctx{h}", name=f"ctx{h}")
                      for h in range(H6)]
            msk_sb = pb.tile([128, 4 * QCW], BF16, tag="masks")
            nc.sync.dma_start(msk_sb[:], t["masks"])
            wo_sb = pb.tile([D, H6 * E], F32R, tag="wo")
            nc.sync.dma_start(wo_sb[:], t["wo"])
            NOB = 384

            for qc in range(NQC):
                nkc = 4 * qc + 4
                for h in range(H6):
                    qh, kh = qk_bf[("q", h)], qk_bf[("k", h)]
                    cps = pb_ps.tile([97, QCW], F32, tag="ctxps", bufs=2)
                    for kp in range(nkc // 2):
                        sps = pb_ps.tile([128, 2 * QCW], F32, tag="sps", bufs=2)
                        for i in range(2):
                            kc = 2 * kp + i
                            nc.tensor.matmul(
                                sps[:, i * QCW:(i + 1) * QCW],
                                kh[:, kc * 128:(kc + 1) * 128],
                                qh[:, qc * QCW:(qc + 1) * QCW],
                                start=True, stop=True)
                        pt = pb.tile([128, 2 * QCW], BF16, tag="pt", bufs=3)
                        nc.scalar.activation(pt[:], sps[:],
                                             mybir.ActivationFunctionType.Exp,
                                             scale=SCALE)
                        for i in range(2):
                            kc = 2 * kp + i
                            j = kc - 4 * qc
                            if j >= 0:  # diagonal-crossing chunk: causal mask
                                nc.vector.tensor_mul(
                                    pt[:, i * QCW:(i + 1) * QCW],
                                    pt[:, i * QCW:(i + 1) * QCW],
                                    msk_sb[:, j * QCW:(j + 1) * QCW])
                        for i in range(2):
                            kc = 2 * kp + i
                            nc.tensor.matmul(
                                cps[:], v_sb[kc][:, h * 97:(h + 1) * 97],
                                pt[:, i * QCW:(i + 1) * QCW],
                                start=(kc == 0), stop=(kc == nkc - 1))
                    # normalize: ctx[0:96] * (1 / ctx[96])
                    rec = pb.tile([1, QCW], F32, tag="rec", bufs=2)
                    with nc.allow_low_precision(reason="softmax reciprocal"):
                        nc.vector.reciprocal(rec[:], cps[96:97, :])
                    rsb = pb.tile([D, QCW], F32, tag="rsb", bufs=2)
                    nc.gpsimd.partition_broadcast(rsb[:], rec[:])
                    nc.vector.tensor_mul(
                        ctx_sb[h][:, qc * QCW:(qc + 1) * QCW], cps[0:96, :], rsb[:])

                # output projection for this q-chunk's token tiles
                for tl in range(4):
                    tt = 4 * qc + tl
                    osb = pb.tile([128, E], F32, tag="osb", bufs=2, name=f"osb{tt}")
                    for i in range(3):
                        ops = pb_ps.tile([128, NOB], F32, tag="ops", bufs=2)
                        for h in range(H6):
                            nc.tensor.matmul(
                                ops[:], ctx_sb[h][:, tt * 128:(tt + 1) * 128],
                                wo_sb[:, h * E + i * NOB:h * E + (i + 1) * NOB],
                                start=(h == 0), stop=(h == H6 - 1))
                        nc.vector.tensor_copy(osb[:, i * NOB:(i + 1) * NOB], ops[:])
                    nc.sync.dma_start(t["o"][tt * 128:(tt + 1) * 128, :], osb[:])


def build_nc(reps=1):
    nc = bacc.Bacc("TRN2", target_bir_lowering=False, debug=False, num_devices=8)
    t = {
        "x": nc.dram_tensor("x", [S, E], F32, kind="ExternalInput").ap(),
        "wq": nc.dram_tensor("wq", [E, H6 * D], F32R, kind="ExternalInput").ap(),
        "wk": nc.dram_tensor("wk", [E, H6 * D], F32R, kind="ExternalInput").ap(),
        "wv": nc.dram_tensor("wv", [E + 1, H6 * 97], F32R, kind="ExternalInput").ap(),
        "wo": nc.dram_tensor("wo", [D, H6 * E], F32R, kind="ExternalInput").ap(),
        "bq": nc.dram_tensor("bq", [D, H6], F32, kind="ExternalInput").ap(),
        "bk": nc.dram_tensor("bk", [D, H6], F32, kind="ExternalInput").ap(),
        "cosext": nc.dram_tensor("cosext", [D, S], F32, kind="ExternalInput").ap(),
        "sinext": nc.dram_tensor("sinext", [D, S], F32, kind="ExternalInput").ap(),
        "masks": nc.dram_tensor("masks", [128, 4 * QCW], BF16,
                                kind="ExternalInput").ap(),
        "o": nc.dram_tensor("o", [S, E], F32, kind="ExternalOutput").ap(),
    }
    with tile.TileContext(nc) as tc:
        for rep in range(reps):
            if rep:
                tc.strict_bb_all_engine_barrier()
            _emit(nc, tc, t, rep)
    nc.compile()
    return nc


_NC = None


def _get_nc():
    global _NC
    if _NC is None:
        _NC = build_nc()
    return _NC


def make_in_maps(logits, Wq, bq, Wk, bk, Wv, bv, Wo, bo):
    """Build the 8 per-core input maps (host-side sharding + preprocessing)."""
    logits = np.asarray(logits, np.float32)
    Wq, Wk, Wv, Wo = (np.asarray(a, np.float32) for a in (Wq, Wk, Wv, Wo))
    bq, bk, bv = (np.asarray(a, np.float32) for a in (bq, bk, bv))

    # head-dim permutation: interleaved pairs -> [even comps | odd comps]
    def perm_w(w):
        return (w.reshape(E, H, D // 2, 2).transpose(0, 1, 3, 2)
                .reshape(E, H * D))

    def perm_b(b):
        return b.reshape(H, D // 2, 2).transpose(0, 2, 1).reshape(H * D)

    wq_p, wk_p = perm_w(Wq), perm_w(Wk)
    bq_p, bk_p = perm_b(bq), perm_b(bk)

    # RoPE tables in [dim, token] layout, rotate-half form
    theta = (1.0 / ROPE_BASE ** (np.arange(0, D, 2, dtype=np.float64) / D))
    ang = np.arange(S, dtype=np.float64)[:, None] * theta[None, :]  # [S, 48]
    cos = np.cos(ang).T.astype(np.float32)  # [48, S]
    sin = np.sin(ang).T.astype(np.float32)
    cosext = np.vstack([cos, cos]).copy()
    sinext = np.vstack([-sin, sin]).copy()

    # causal masks for the 4 diagonal-crossing chunk offsets
    p = np.arange(128)[:, None]
    f = np.arange(QCW)[None, :]
    masks = np.concatenate(
        [(p <= f - 128 * j).astype(ml_dtypes.bfloat16) for j in range(4)], axis=1)

    in_maps = []
    for c in range(8):
        b_i = c // 2
        h0 = (c % 2) * H6
        cs, ce = h0 * D, (h0 + H6) * D

        wvp = np.zeros((E + 1, H6 * 97), np.float32)
        for hh in range(H6):
            g = (h0 + hh) * D
            wvp[:E, 97 * hh:97 * hh + D] = Wv[:, g:g + D]
            wvp[E, 97 * hh:97 * hh + D] = bv[g:g + D]
            wvp[E, 97 * hh + D] = 1.0

        wo_s = (Wo[cs:ce].reshape(H6, D, E).transpose(1, 0, 2)
                .reshape(D, H6 * E)).copy()

        in_maps.append({
            "x": np.ascontiguousarray(logits[b_i]),
            "wq": np.ascontiguousarray(wq_p[:, cs:ce]),
            "wk": np.ascontiguousarray(wk_p[:, cs:ce]),
            "wv": wvp,
            "wo": wo_s,
            "bq": np.ascontiguousarray(bq_p[cs:ce].reshape(H6, D).T),
            "bk": np.ascontiguousarray(bk_p[cs:ce].reshape(H6, D).T),
            "cosext": cosext,
            "sinext": sinext,
            "masks": masks,
        })
    return in_maps


def assemble_output(results, bo):
    bo = np.asarray(bo, np.float32)
    out = np.empty((B, S, E), np.float32)
    for b_i in range(B):
        out[b_i] = results[2 * b_i]["o"] + results[2 * b_i + 1]["o"] + bo
    return out


def kernel(logits, Wq, bq, Wk, bk, Wv, bv, Wo, bo, batch_size, seq_len):
    assert int(batch_size) == B and int(seq_len) == S
    nc = _get_nc()
    in_maps = make_in_maps(logits, Wq, bq, Wk, bk, Wv, bv, Wo, bo)
    res = run_bass_kernel_spmd(nc, in_maps, core_ids=list(range(8)))
    return assemble_output(res.results, bo)
